# revision 11
# baseline (speedup 1.0000x reference)
"""Trainium2 Bass kernel for nn_Linear_27608049779368 (lean body).

Reference computation:
    out[b,c] = bias[c] + sum_o prod(x[:, idx_o], axis=2) @ W_o
    x [4096, 32], orders 1..3 with 32/496/4960 combos, C=128 classes.

Device algorithm (per core, data-parallel over batch, 8 cores x 512 rows):
    out.T = Wp.T @ exp(Inc.T @ log(x.T + c))        (fp32 / fp32r)

v2 vs baseline: minimal instruction & DMA count per body.
  * 4 input DMAs total; Wp shipped pre-transposed so it is one contiguous
    [128, nt*128] HWDGE transfer (128 x 22 KB lines).
  * Prep is ONE activation: lx = Ln(x + c) (bias folded into the ACT op,
    the max(.,1/64) clamp never binds since min(x+c) = 0.5), written
    straight to an fp32r tile (rounds to 11 mantissa bits).
  * Both matmul stages run fp32r (single pass each).  Per-feature shifts
    c_f = 0.5 - min_b(x[:,f]) keep the product magnitudes small enough
    that the 11-bit operand rounding lands at 1.34e-2 rel error on HW
    (budget 2e-2); a global shift would be 2.5e-2.
  * No anti-mean / big-row-split machinery (fp32 PSUM accumulation
    without it models at ~1.2e-3 rel).
  * exp fused 4 tiles per ACTIVATE ([128, 2048] PSUM->SBUF).
  * Main contraction plain fp32 (17-bit effective products).
  * PSUM->SBUF evacuation on ScalarE (Copy), not DVE.
"""

import os
import sys

import numpy as np

for _p in ("/opt/trn_rl_repo", "/root/.axon_site/_ro/trn_rl_repo"):
    if os.path.isdir(_p) and _p not in sys.path:
        sys.path.insert(0, _p)
        break

import concourse.bass as bass
import concourse.bacc as bacc
import concourse.tile as tile
from concourse import mybir
from concourse.bass_utils import run_bass_kernel_spmd

N_CORES = 8
P = 128
EXP_FUSE = 7            # k-tiles per fused exp op (7 PSUM banks + 1 out bank)
NEWTON = False          # one Newton step on Ln (3 extra ops) if needed
F32 = mybir.dt.float32
F32R = mybir.dt.float32r


# ----------------------------------------------------------------------------
# Host-side math: rows, incidence, transformed weights
# ----------------------------------------------------------------------------

def _build_rows(idx_list, W_list, bias, c, F=32):
    """Row table (multisets), incidence Inc [F, NK] and transformed weights
    Wp [NK, C] (f64) such that  out = Wp.T @ exp(Inc.T @ log(x + c))."""
    from itertools import combinations as _comb
    C = W_list[0].shape[1]
    row_of = {}
    rows = []

    def get_row(t):
        r = row_of.get(t)
        if r is None:
            r = len(rows)
            row_of[t] = r
            rows.append(t)
        return r

    for idx, W in zip(idx_list, W_list):
        for k in range(idx.shape[0]):
            get_row(tuple(sorted(int(v) for v in idx[k])))

    Wp_contrib = []
    const_acc = np.array(bias, np.float64).reshape(-1).copy()
    cf = np.asarray(c, np.float64).reshape(-1)
    for idx, W in zip(idx_list, W_list):
        o = idx.shape[1]
        for k in range(idx.shape[0]):
            M = tuple(sorted(int(v) for v in idx[k]))
            Wk = W[k].astype(np.float64)
            for r in range(o, -1, -1):
                for sub in _comb(M, r):
                    # all index sets are distinct combinations, so the
                    # expansion coefficient is just the product of the
                    # per-feature shifts of the removed features
                    sset = set(sub)
                    coeff = 1.0
                    for f in M:
                        if f not in sset:
                            coeff *= -cf[f]
                    if r == 0:
                        const_acc += coeff * Wk
                    else:
                        Wp_contrib.append((get_row(tuple(sub)), coeff, Wk))

    const_row = get_row(())
    NK = len(rows)
    Inc = np.zeros((F, NK), np.float32)
    for r, t in enumerate(rows):
        for f in t:
            Inc[f, r] += 1.0
    Wp = np.zeros((NK, C), np.float64)
    for r, coeff, Wk in Wp_contrib:
        Wp[r] += coeff * Wk
    Wp[const_row] += const_acc
    return Inc, Wp


def _prepare(x, bias, W1, W2, W3, idx1, idx2, idx3):
    x = np.asarray(x)
    F = x.shape[1]
    C = np.asarray(W1).shape[1]
    # per-feature shifts keep the exp-domain product magnitudes ~2.4x
    # smaller than a global shift, which is what lets the main contraction
    # run in fp32r (11-bit operands) within the 2e-2 error budget.
    c = np.maximum(1.0, 0.5 - x.min(axis=0).astype(np.float64))
    Inc, Wp = _build_rows(
        [np.asarray(idx1), np.asarray(idx2), np.asarray(idx3)],
        [np.asarray(W1), np.asarray(W2), np.asarray(W3)],
        np.asarray(bias), c, F=F)
    NK = Inc.shape[1]
    nt = -(-NK // P)
    pad = nt * P - NK
    if pad:
        Inc = np.concatenate([Inc, np.zeros((F, pad), np.float32)], axis=1)
        Wp = np.concatenate([Wp, np.zeros((pad, Wp.shape[1]), np.float64)], axis=0)
    # Wp packed so the whole thing is one [128, nt*128+1] contiguous DMA:
    # partition p, block t, col j  =  Wp[t*128 + p, j].  The final column is
    # zero and serves as the Exp activation's bias AP (avoids the const-AP
    # memset + init barrier).
    WpA = np.ascontiguousarray(np.concatenate([
        Wp.astype(np.float32).reshape(nt, P, C).transpose(1, 0, 2).reshape(P, nt * C),
        np.zeros((P, 1), np.float32)], axis=1))
    return c, np.ascontiguousarray(Inc), WpA, nt


# ----------------------------------------------------------------------------
# Device kernel
# ----------------------------------------------------------------------------

def _body_once(nc, tc, consts, prods_pool, psum_L, psum_out,
               d_outT, x_sb, inc_sb, wp_sb, F, C, b_shard, nt):
    # x_sb already holds x + c (host-shifted); min = 0.5 so no clamp.
    lx = consts.tile([F, b_shard], F32R)
    zb_x = x_sb[:, b_shard:b_shard + 1]      # shipped zero column
    zb_w = wp_sb[:, nt * C:nt * C + 1]       # shipped zero column
    if not NEWTON:
        nc.scalar.activation(lx, x_sb[:, :b_shard],
                             mybir.ActivationFunctionType.Ln, bias=zb_x)
    else:
        lx0 = consts.tile([F, b_shard], F32)
        nc.scalar.activation(lx0, x_sb, mybir.ActivationFunctionType.Ln)
        e_neg = consts.tile([F, b_shard], F32)
        nc.scalar.activation(e_neg, lx0, mybir.ActivationFunctionType.Exp,
                             scale=-1.0)
        corr = consts.tile([F, b_shard], F32)
        nc.vector.tensor_mul(out=corr, in0=x_sb, in1=e_neg)
        nc.vector.scalar_tensor_tensor(
            out=lx, in0=corr, scalar=1.0, in1=lx0,
            op0=mybir.AluOpType.subtract, op1=mybir.AluOpType.add)

    out_ps = psum_out.tile([C, b_shard], F32)
    G = -(-nt // EXP_FUSE)
    for g in range(G):
        w = min(EXP_FUSE, nt - g * EXP_FUSE)
        L_ps = psum_L.tile([P, EXP_FUSE * b_shard], F32, tag="L")
        for k in range(w):
            t = g * EXP_FUSE + k
            nc.tensor.matmul(
                L_ps[:, k * b_shard:(k + 1) * b_shard],
                inc_sb[:, t * P:(t + 1) * P],
                lx,
                start=True, stop=True)
        pg = prods_pool.tile([P, EXP_FUSE * b_shard], F32R, tag="pg")
        nc.scalar.activation(
            pg[:, :w * b_shard], L_ps[:, :w * b_shard],
            mybir.ActivationFunctionType.Exp, bias=zb_w)
        for k in range(w):
            t = g * EXP_FUSE + k
            nc.tensor.matmul(
                out_ps, wp_sb[:, t * C:(t + 1) * C],
                pg[:, k * b_shard:(k + 1) * b_shard],
                start=(t == 0), stop=(t == nt - 1))

    out_sb = consts.tile([C, b_shard], F32)
    nc.scalar.activation(out_sb, out_ps, mybir.ActivationFunctionType.Copy)
    nc.sync.dma_start(out=d_outT[:, :], in_=out_sb)


_nc_cache = {}


def _get_nc(F, C, b_shard, nt, c, repeat=1):
    key = (F, C, b_shard, nt, repeat)
    if key not in _nc_cache:
        _nc_cache[key] = _build_nc_with_c(F, C, b_shard, nt, c, repeat)
    return _nc_cache[key]


def _build_nc_with_c(F, C, b_shard, nt, c, repeat):
    nc = bacc.Bacc(None, target_bir_lowering=False)
    # inc carries [Inc | x'+c | zero-col] so input staging is one DMA fewer;
    # the x' slice is read back as plain f32 via bitcast (DMA is a byte copy).
    d_inc = nc.declare_dram_parameter(
        "inc", [F, nt * P + b_shard + 1], F32R, isOutput=False)
    d_wp = nc.declare_dram_parameter("wp", [P, nt * C + 1], F32R, isOutput=False)
    d_outT = nc.declare_dram_parameter("outT", [C, b_shard], F32, isOutput=True)

    with tile.TileContext(nc) as tc:
        with (
            tc.tile_pool(name="consts", bufs=1) as consts,
            tc.tile_pool(name="prods", bufs=7) as prods_pool,
            tc.tile_pool(name="psum_L", bufs=1, space="PSUM") as psum_L,
            tc.tile_pool(name="psum_out", bufs=1, space="PSUM") as psum_out,
        ):
            inc_sb = consts.tile([F, nt * P + b_shard + 1], F32R)
            nc.sync.dma_start(out=inc_sb, in_=d_inc[:, :])
            x_sb = inc_sb[:, nt * P:nt * P + b_shard + 1].bitcast(F32)
            wp_sb = consts.tile([P, nt * C + 1], F32R)
            nc.sync.dma_start(out=wp_sb, in_=d_wp[:, :])

            for _rep in range(repeat):
                _body_once(nc, tc, consts, prods_pool, psum_L, psum_out,
                           d_outT, x_sb, inc_sb, wp_sb, F, C, b_shard, nt)
    nc.finalize()
    _merge_act_table_loads(nc)
    _strip_overhead(nc)
    return nc


def _strip_overhead(nc):
    """Drop setup/tail instructions that don't affect this kernel's result:
    the unused const-AP memsets (only the f32 0.0 bias constant is read)
    and the end-block all-engine barrier cascade (drains / event semaphores
    that reference only barrier_* rendezvous semaphores).  The data-bearing
    completion waits (DMA / engine sems) are kept, so the output DMA is
    still guaranteed complete at NEFF end.  Validated under CoreSim's race
    detector."""
    blocks = list(nc.m.functions[0].blocks)
    for b in blocks:
        keep = []
        for inst in b.instructions:
            nm = type(inst).__name__
            si = inst.sync_info
            sems = []
            if si is not None:
                sems += [w.ant_name for w in si.on_wait]
                sems += [u.ant_name for u in si.on_update]
            only_barrier = bool(sems) and all(
                s.startswith("barrier_") for s in sems)
            if nm == "InstMemset" and "const-" in str(
                    getattr(inst.outs[0], "memref", "")):
                continue
            if nm in ("InstDrain", "InstEventSemaphore") and (
                    only_barrier or (nm == "InstDrain" and not sems)):
                continue
            # the remaining SP drain (wait ACT==all) is subsumed by the
            # out-DMA completion waits; InstISA is a Pool marker and no
            # Pool work remains after the memset removal
            if nm == "InstISA":
                continue
            if nm == "InstDrain":
                continue
            keep.append(inst)
        b.instructions[:] = keep
    # standalone DMA-completion waits are droppable when every sem they
    # wait on retains another waiter among the REMAINING instructions:
    # ordering then follows transitively through that waiter's consumers.
    blocks = list(nc.m.functions[0].blocks)
    insts = [i for b in blocks for i in b.instructions]
    dma_sems = set()
    for i in insts:
        if type(i).__name__ == "InstDMACopy" and i.sync_info:
            dma_sems |= {u.ant_name for u in i.sync_info.on_update}
    waiters = {}
    for i in insts:
        if i.sync_info:
            for w in i.sync_info.on_wait:
                waiters.setdefault(w.ant_name, set()).add(id(i))
    dropped = set()
    for i in reversed(insts):
        if (type(i).__name__ == "InstEventSemaphore" and i.sync_info
                and i.sync_info.on_wait and not i.sync_info.on_update):
            names = {w.ant_name for w in i.sync_info.on_wait}
            if names <= dma_sems and all(
                    len(waiters[n] - dropped - {id(i)}) >= 1 for n in names):
                dropped.add(id(i))
    for b in blocks:
        b.instructions[:] = [i for i in b.instructions if id(i) not in dropped]
    # merge the (now purely sequential) blocks into one and drop the
    # inter-block branches; per-engine instruction order is preserved
    merged = []
    for b in blocks:
        for inst in b.instructions:
            if type(inst).__name__ == "InstUnconditionalBranch":
                continue
            merged.append(inst)
    blocks[0].instructions[:] = merged
    nc.m.functions[0].blocks[:] = [blocks[0]]


def _merge_act_table_loads(nc):
    """All activations here (Ln, Exp, Copy) live in one table set
    (natural_log_exp_and_others); keep a single load of that set instead
    of the per-function flip-flop bacc emits."""
    from concourse.hw_specs import get_activation_tables
    T = mybir.ActivationFunctionType
    tabs = get_activation_tables(nc.m.arch)
    combined = None
    for i, fns in enumerate(tabs.values()):
        if {T.Ln, T.Exp, T.Copy} <= fns:
            combined = i
            break
    if combined is None:
        return
    first = True
    for b in nc.m.functions[0].blocks:
        keep = []
        for inst in b.instructions:
            if isinstance(inst, mybir.InstLoadActFuncSet):
                if first:
                    inst.act_func_set_id = combined
                    first = False
                    keep.append(inst)
                # later loads dropped: set already resident
            else:
                keep.append(inst)
        b.instructions[:] = keep


def _make_in_maps(x, c, Inc, WpA, b_shard):
    F = x.shape[1]
    in_maps = []
    for i in range(N_CORES):
        shift = np.asarray(c, np.float64).reshape(1, -1)
        sh = np.ascontiguousarray(np.concatenate([
            Inc,
            (x[i * b_shard:(i + 1) * b_shard].astype(np.float64) + shift)
            .T.astype(np.float32),
            np.zeros((F, 1), np.float32)], axis=1))
        in_maps.append({"inc": sh, "wp": WpA})
    return in_maps


def kernel(x, bias, W1, W2, W3, idx1, idx2, idx3, _trace=False):
    x = np.asarray(x, np.float32)
    B, F = x.shape
    C = np.asarray(W1).shape[1]
    assert B % N_CORES == 0
    b_shard = B // N_CORES

    c, Inc, WpA, nt = _prepare(x, bias, W1, W2, W3, idx1, idx2, idx3)
    nc = _get_nc(F, C, b_shard, nt, c)
    in_maps = _make_in_maps(x, c, Inc, WpA, b_shard)
    res = run_bass_kernel_spmd(nc, in_maps, list(range(N_CORES)), trace=_trace)
    out = np.empty((B, C), np.float32)
    for i in range(N_CORES):
        out[i * b_shard:(i + 1) * b_shard] = res.results[i]["outT"].T
    if _trace:
        kernel.last_results = res
    return out
